# revision 3
# baseline (speedup 1.0000x reference)
"""MoE-GAT kernel for Trainium2 (Bass/Tile), SPMD over 8 NeuronCores.

Sharding: data-parallel over batch (B=8 -> 1 batch element per core).
Each core receives its x[b] / adj[b] slice plus the full shared weights
and computes out[b] = MoEGAT(x[b], adj[b]) independently (no collectives).

Math per core (N=1024 nodes, D=512 hidden, E=8 experts):
  gate = softmax(x @ gate_W + gate_b)                  [N, E]
  h_e  = x @ W[e]                                      [N, D]
  s_src = h_e @ a_src[e] ; s_dst = h_e @ a_dst[e]      [N]
  scoresT[j, i] = leaky_relu(s_src[i] + s_dst[j], .2)  (transposed layout)
  pmT[j, i] = exp(scoresT) * adj[i, j]
  out_e[i, :] = (pmT[:, i] . h_e) / S_i ; S_i = sum_j pmT[j, i]
  out[i] = sum_e gate[i, e] * elu(out_e[i]) = acc - 1, using sum_e gate = 1
  elu(v) = relu(v) + exp(min(v, 0)) - 1

All big matmuls run in bf16 (1 cycle/row on the PE vs 4 for fp32);
the score path (s = x @ (W @ a)) is bf16 against an fp32 c-pair, which
keeps |score| error ~4e-3 - well within the 2e-2 gate.
"""

import sys

import numpy as np

for _p in ("/opt/trn_rl_repo",):
    if _p not in sys.path:
        sys.path.append(_p)

B, N, D, E = 8, 1024, 512, 8
P = 128
NB = N // P  # 8 node blocks
DB = D // P  # 4 hidden blocks
SLOPE = 0.2

_CACHE = {}


def _build():
    from contextlib import ExitStack

    import concourse.bass as bass
    import concourse.tile as tile
    from concourse import bacc, mybir
    from concourse.masks import make_identity

    f32 = mybir.dt.float32
    bf16 = mybir.dt.bfloat16
    i32 = mybir.dt.int32
    AF = mybir.ActivationFunctionType
    OP = mybir.AluOpType
    ts = bass.ts

    nc = bacc.Bacc("TRN2", target_bir_lowering=False, debug=False, num_swdge_queues=4)

    x_d = nc.dram_tensor("x", [N, D], f32, kind="ExternalInput")
    adj_d = nc.dram_tensor("adj", [N, N], i32, kind="ExternalInput")
    gw_d = nc.dram_tensor("gate_W", [D, E], f32, kind="ExternalInput")
    gb_d = nc.dram_tensor("gate_b", [E], f32, kind="ExternalInput")
    W_d = nc.dram_tensor("W", [E, D, D], f32, kind="ExternalInput")
    asrc_d = nc.dram_tensor("a_src", [E, D], f32, kind="ExternalInput")
    adst_d = nc.dram_tensor("a_dst", [E, D], f32, kind="ExternalInput")
    out_d = nc.dram_tensor("out", [N, D], f32, kind="ExternalOutput")

    with tile.TileContext(nc) as tc, ExitStack() as ctx:
        # persistent pools
        const = ctx.enter_context(tc.tile_pool(name="const", bufs=1))
        xT_p = ctx.enter_context(tc.tile_pool(name="xT", bufs=1))
        adjT_p = ctx.enter_context(tc.tile_pool(name="adjT", bufs=1))
        gate_p = ctx.enter_context(tc.tile_pool(name="gate", bufs=1))
        acc_p = ctx.enter_context(tc.tile_pool(name="acc", bufs=1))
        ps_big = ctx.enter_context(tc.tile_pool(name="ps_big", bufs=4, space="PSUM"))
        ps_bc = ctx.enter_context(tc.tile_pool(name="ps_bc", bufs=2, space="PSUM"))

        ident = const.tile([P, P], f32)
        make_identity(nc, ident)
        ident_b = const.tile([P, P], bf16)
        nc.vector.tensor_copy(out=ident_b, in_=ident)
        ones_col_b = const.tile([P, 1], bf16)
        nc.vector.memset(ones_col_b, 1.0)
        ones_row = const.tile([1, P], f32)
        nc.vector.memset(ones_row, 1.0)
        ones_row_b = const.tile([1, P], bf16)
        nc.vector.memset(ones_row_b, 1.0)

        gwsb = const.tile([P, DB, E], f32)
        nc.sync.dma_start(gwsb, gw_d.ap().rearrange("(db p) e -> p db e", p=P))
        gwsb_b = const.tile([P, DB, E], bf16)
        nc.vector.tensor_copy(out=gwsb_b, in_=gwsb)
        gbsb = const.tile([1, E], f32)
        nc.sync.dma_start(gbsb, gb_d.ap().rearrange("(o e) -> o e", o=1))
        gbsb_b = const.tile([1, E], bf16)
        nc.vector.tensor_copy(out=gbsb_b, in_=gbsb)

        xT_b = xT_p.tile([P, DB, N], bf16)  # xT_b[p, db, n] = x[n, db*128+p]
        adjT = adjT_p.tile([P, NB, N], bf16)  # adjT[p, jb, i] = adj[i, jb*128+p]
        gate = gate_p.tile([P, NB, E], f32)  # gate[p, nb, e]
        acc_t = acc_p.tile([P, NB, D], f32)  # MoE-combine accumulator

        # ---- stage 0: x transpose, gate softmax, adj transpose -------------
        with (
            tc.tile_pool(name="xin", bufs=3) as xin,
            tc.tile_pool(name="adjin", bufs=2) as adjin,
            tc.tile_pool(name="adjf", bufs=2) as adjf_p,
        ):
            for nb in range(NB):
                xt = xin.tile([P, D], f32, tag="xin")
                nc.sync.dma_start(xt, x_d[ts(nb, P), :])
                xt_b = xin.tile([P, D], bf16, tag="xin_b")
                if nb % 2 == 0:
                    nc.vector.tensor_copy(out=xt_b, in_=xt)
                else:
                    nc.scalar.copy(out=xt_b, in_=xt)
                pst = ps_big.tile([P, 512], bf16, tag="ps")
                for db in range(DB):
                    nc.tensor.transpose(pst[:, ts(db, P)], xt_b[:, ts(db, P)], ident_b)
                src = pst[:, 0:D].rearrange("p (db q) -> p db q", q=P)
                nc.vector.tensor_copy(out=xT_b[:, :, ts(nb, P)], in_=src)

            for nb in range(NB):
                psg = ps_big.tile([P, 512], f32, tag="ps")
                for db in range(DB):
                    nc.tensor.matmul(
                        psg[:, 0:E],
                        xT_b[:, db, ts(nb, P)],
                        gwsb_b[:, db, :],
                        start=(db == 0),
                        stop=False,
                    )
                nc.tensor.matmul(psg[:, 0:E], ones_row_b, gbsb_b, start=False, stop=True)
                eg = const.tile([P, E], f32, tag=f"eg{nb % 2}")
                sg = const.tile([P, 1], f32, tag=f"sg{nb % 2}")
                nc.scalar.activation(eg, psg[:, 0:E], AF.Exp, accum_out=sg)
                rg0 = const.tile([P, 1], f32, tag=f"rg0{nb % 2}")
                nc.vector.reciprocal(rg0, sg)
                nc.vector.tensor_scalar(gate[:, nb, :], eg, rg0, None, OP.mult)

            for ib in range(NB):
                at = adjin.tile([P, N], i32, tag="adjin")
                nc.sync.dma_start(at, adj_d[ts(ib, P), :])
                af = adjf_p.tile([P, N], bf16, tag="adjf")
                if ib % 2 == 0:
                    nc.vector.tensor_copy(out=af, in_=at)
                else:
                    nc.scalar.copy(out=af, in_=at)
                for half in range(2):
                    pst = ps_big.tile([P, 512], bf16, tag="ps")
                    for k in range(4):
                        jb = half * 4 + k
                        nc.tensor.transpose(pst[:, ts(k, P)], af[:, ts(jb, P)], ident_b)
                    src = pst.rearrange("p (jb q) -> p jb q", q=P)
                    dst = adjT[:, half * 4 : (half + 1) * 4, ts(ib, P)]
                    nc.vector.tensor_copy(out=dst, in_=src)

        # ---- expert loop ----------------------------------------------------
        with (
            tc.tile_pool(name="W", bufs=2) as W_p,
            tc.tile_pool(name="apair", bufs=2) as apair_p,
            tc.tile_pool(name="h", bufs=3) as h_p,
            tc.tile_pool(name="pmT", bufs=12) as pmT_p,
            tc.tile_pool(name="tsc", bufs=2) as t_p,
            tc.tile_pool(name="psc", bufs=3) as p_p,
            tc.tile_pool(name="scol", bufs=2) as scol_p,
            tc.tile_pool(name="srow", bufs=2) as srow_p,
            tc.tile_pool(name="elu", bufs=2) as elu_p,
            tc.tile_pool(name="tiny", bufs=4) as tiny_p,
            tc.tile_pool(name="outw", bufs=2) as outw_p,
        ):
            for e in range(E):
                Wt = W_p.tile([P, DB, D], f32, tag="W")
                nc.sync.dma_start(Wt, W_d[e].rearrange("(db p) z -> p db z", p=P))
                Wt_b = W_p.tile([P, DB, D], bf16, tag="Wb")
                for db in range(DB):
                    if db % 2 == 0:
                        nc.vector.tensor_copy(out=Wt_b[:, db, :], in_=Wt[:, db, :])
                    else:
                        nc.scalar.copy(out=Wt_b[:, db, :], in_=Wt[:, db, :])
                arow = apair_p.tile([1, 2 * D], f32, tag="arow")
                nc.sync.dma_start(arow[:, 0:D], asrc_d[e].rearrange("(o z) -> o z", o=1))
                nc.sync.dma_start(
                    arow[:, D : 2 * D], adst_d[e].rearrange("(o z) -> o z", o=1)
                )
                # broadcast a_src/a_dst across partitions (GPSIMD), then
                # c_pair[d, v] = sum_z W[d, z] * a_v[z]  (GPSIMD mult + DVE reduce)
                # so that s_v = x @ c_v = (x @ W) @ a_v = h @ a_v.
                absf = apair_p.tile([P, 2 * D], f32, tag="absf")
                nc.gpsimd.partition_broadcast(absf, arow)
                ap_t = apair_p.tile([P, DB, 2], f32, tag="cpair")
                for db in range(DB):
                    for v in range(2):
                        scr = apair_p.tile([P, D], f32, tag="cw_scratch")
                        nc.gpsimd.tensor_mul(scr, Wt[:, db, :], absf[:, v * D : (v + 1) * D])
                        nc.vector.reduce_sum(
                            ap_t[:, db, v : v + 1], scr, axis=mybir.AxisListType.X
                        )
                ap_t_b = apair_p.tile([P, DB, 2], bf16, tag="cpair_b")
                nc.vector.tensor_copy(out=ap_t_b, in_=ap_t)

                # s rows: s_row[v, n] = sum_d c_pair[d, v] * x[n, d]  (bf16)
                psrc = ps_bc.tile([P, N], f32, tag="bc")
                psdst = ps_bc.tile([P, N], f32, tag="bc")
                for v, psv in ((0, psrc), (1, psdst)):
                    for half in range(2):
                        for db in range(DB):
                            nc.tensor.matmul(
                                psv[0:1, half * 512 : (half + 1) * 512],
                                ap_t_b[:, db, v : v + 1],
                                xT_b[:, db, half * 512 : (half + 1) * 512],
                                start=(db == 0),
                                stop=(db == DB - 1),
                            )
                ssrow = srow_p.tile([1, N], bf16, tag="ssrow")
                nc.vector.tensor_copy(out=ssrow, in_=psrc[0:1, :])
                sdrow = srow_p.tile([1, N], bf16, tag="sdrow")
                nc.scalar.copy(out=sdrow, in_=psdst[0:1, :])
                # s_dst to per-partition column form [128, NB] via tiny matmuls
                psd = ps_big.tile([P, 512], f32, tag="ps")
                for nb in range(NB):
                    nc.tensor.matmul(
                        psd[:, nb : nb + 1],
                        sdrow[0:1, ts(nb, P)],
                        ones_row_b[0:1, 0:1],
                        start=True,
                        stop=True,
                    )
                sdcol = scol_p.tile([P, NB], f32, tag="sdcol")
                nc.vector.tensor_copy(out=sdcol, in_=psd[:, 0:NB])
                # broadcast s_src across partitions via K=1 matmuls; the
                # Prelu reads the result straight from PSUM (no SBUF copy)
                bc = ps_bc.tile([P, N], f32, tag="bc")
                nc.tensor.matmul(
                    bc[:, 0:512], ones_row_b, ssrow[:, 0:512], start=True, stop=True
                )
                nc.tensor.matmul(
                    bc[:, 512:1024], ones_row_b, ssrow[:, 512:1024], start=True, stop=True
                )

                # scores -> exp -> mask, in [j, i] layout
                pm_tiles = []
                for jb in range(NB):
                    tsc = t_p.tile([P, N], f32, tag="t")
                    nc.scalar.activation(
                        tsc, bc, AF.Prelu, bias=sdcol[:, jb : jb + 1], alpha=SLOPE
                    )
                    psc = p_p.tile([P, N], bf16, tag="p")
                    nc.scalar.activation(psc, tsc, AF.Exp)
                    pm = pmT_p.tile([P, N], bf16, tag="pmT")
                    nc.vector.tensor_mul(pm, psc, adjT[:, jb, :])
                    pm_tiles.append(pm)

                # h in bf16: feeds only the (bf16) attention matmul; s/scores
                # come from the fp32 c_pair path so accuracy is preserved.
                h_t = h_p.tile([P, NB, D], bf16, tag="h")
                for nb in range(NB):
                    ph = ps_big.tile([P, 512], f32, tag="ps")
                    for db in range(DB):
                        nc.tensor.matmul(
                            ph,
                            xT_b[:, db, ts(nb, P)],
                            Wt_b[:, db, :],
                            start=(db == 0),
                            stop=(db == DB - 1),
                        )
                    if nb % 2 == 0:
                        nc.vector.tensor_copy(out=h_t[:, nb, :], in_=ph)
                    else:
                        nc.scalar.copy(out=h_t[:, nb, :], in_=ph)

                # softmax denominators S_i = sum_j pmT[j, i], as a row via
                # ones-column stationary matmuls accumulating over j blocks
                psS = ps_bc.tile([P, N], f32, tag="bc")
                for half in range(2):
                    for jb in range(NB):
                        nc.tensor.matmul(
                            psS[0:1, half * 512 : (half + 1) * 512],
                            ones_col_b,
                            pm_tiles[jb][:, half * 512 : (half + 1) * 512],
                            start=(jb == 0),
                            stop=(jb == NB - 1),
                        )
                Srow = srow_p.tile([1, N], bf16, tag="Srow")
                nc.scalar.copy(out=Srow, in_=psS[0:1, :])
                psc2 = ps_big.tile([P, 512], f32, tag="ps")
                for nb in range(NB):
                    nc.tensor.matmul(
                        psc2[:, nb : nb + 1],
                        Srow[0:1, ts(nb, P)],
                        ones_row_b[0:1, 0:1],
                        start=True,
                        stop=True,
                    )
                rS8 = tiny_p.tile([P, NB], f32, tag="rS8")
                nc.vector.reciprocal(rS8, psc2[:, 0:NB])

                # attention matmul + ELU + gated combine
                for ib in range(NB):
                    po = ps_big.tile([P, 512], f32, tag="ps")
                    for jb in range(NB):
                        nc.tensor.matmul(
                            po,
                            pm_tiles[jb][:, ts(ib, P)],
                            h_t[:, jb, :],
                            start=(jb == 0),
                            stop=(jb == NB - 1),
                        )
                    rg = tiny_p.tile([P, 1], f32, tag="rg")
                    nc.vector.tensor_scalar(
                        rg, rS8[:, ib : ib + 1], gate[:, ib, e : e + 1], None, OP.mult
                    )

                    r_sb = elu_p.tile([P, D], f32, tag="r")
                    nc.scalar.activation(r_sb, po, AF.Relu, scale=rg)
                    m_sb = elu_p.tile([P, D], f32, tag="n")
                    nc.vector.tensor_scalar(
                        m_sb, po, rS8[:, ib : ib + 1], 0.0, OP.mult, OP.min
                    )
                    e_sb = elu_p.tile([P, D], f32, tag="e2")
                    nc.scalar.activation(e_sb, m_sb, AF.Exp)

                    if e == 0:
                        # acc = g*e2 + r   (r already carries g via the rg scale)
                        nc.vector.scalar_tensor_tensor(
                            out=acc_t[:, ib, :],
                            in0=e_sb,
                            scalar=gate[:, ib, e : e + 1],
                            in1=r_sb,
                            op0=OP.mult,
                            op1=OP.add,
                        )
                    else:
                        nc.vector.scalar_tensor_tensor(
                            out=acc_t[:, ib, :],
                            in0=e_sb,
                            scalar=gate[:, ib, e : e + 1],
                            in1=acc_t[:, ib, :],
                            op0=OP.mult,
                            op1=OP.add,
                        )
                        nc.vector.tensor_add(acc_t[:, ib, :], acc_t[:, ib, :], r_sb)

            # ---- writeback ---------------------------------------------------
            for ib in range(NB):
                ow = outw_p.tile([P, D], f32, tag="ow")
                nc.vector.tensor_scalar(ow, acc_t[:, ib, :], 1.0, None, OP.subtract)
                nc.sync.dma_start(out_d[ts(ib, P), :], ow)

    nc.compile()
    return nc


def _get_nc():
    if "nc" not in _CACHE:
        _CACHE["nc"] = _build()
    return _CACHE["nc"]


def _reset_device():
    # Defensive: clear any wedged accelerator state left by a prior process.
    try:
        import ctypes

        import jax

        jax.devices()
        lib = ctypes.CDLL("/opt/axon/libaxon_pjrt.so")
        lib.axon_reset.restype = ctypes.c_int64
        lib.axon_reset()
    except Exception:
        pass


def _run(inputs: dict, trace: bool = False):
    from concourse.bass_utils import run_bass_kernel_spmd

    _reset_device()
    nc = _get_nc()
    in_maps = []
    for c in range(8):
        in_maps.append(
            {
                "x": np.ascontiguousarray(inputs["x"][c], dtype=np.float32),
                "adj": np.ascontiguousarray(inputs["adj"][c], dtype=np.int32),
                "gate_W": np.ascontiguousarray(inputs["gate_W"], dtype=np.float32),
                "gate_b": np.ascontiguousarray(inputs["gate_b"], dtype=np.float32),
                "W": np.ascontiguousarray(inputs["W"], dtype=np.float32),
                "a_src": np.ascontiguousarray(inputs["a_src"], dtype=np.float32),
                "a_dst": np.ascontiguousarray(inputs["a_dst"], dtype=np.float32),
            }
        )
    res = run_bass_kernel_spmd(nc, in_maps, list(range(8)), trace=trace)
    out = np.stack([res.results[c]["out"] for c in range(8)], axis=0)
    return out.astype(np.float32), res


def kernel(**inputs) -> np.ndarray:
    out, _ = _run(inputs, trace=False)
    return out


def kernel_traced(**inputs):
    out, res = _run(inputs, trace=True)
    try:
        if res.instructions_and_trace:
            print("trace path:", res.instructions_and_trace[1])
        if res.profile_json:
            print("profile json:", res.profile_json)
    except Exception:
        pass
    return out, res.exec_time_ns


# revision 4
# speedup vs baseline: 1.0907x; 1.0907x over previous
"""MoE-GAT kernel for Trainium2 (Bass/Tile), SPMD over 8 NeuronCores.

Sharding: data-parallel over batch (B=8 -> 1 batch element per core).
Each core receives its x[b] / adj[b] slice plus the full shared weights
and computes out[b] = MoEGAT(x[b], adj[b]) independently (no collectives).

Math per core (N=1024 nodes, D=512 hidden, E=8 experts):
  gate = softmax(x @ gate_W + gate_b)                  [N, E]
  h_e  = x @ W[e]                                      [N, D]
  s_src = h_e @ a_src[e] ; s_dst = h_e @ a_dst[e]      [N]
  scoresT[j, i] = leaky_relu(s_src[i] + s_dst[j], .2)  (transposed layout)
  pmT[j, i] = exp(scoresT) * adj[i, j]
  out_e[i, :] = (pmT[:, i] . h_e) / S_i ; S_i = sum_j pmT[j, i]
  out[i] = sum_e gate[i, e] * elu(out_e[i]) = acc - 1, using sum_e gate = 1
  elu(v) = relu(v) + exp(min(v, 0)) - 1

Software pipeline (per iteration e): S(e) -> prep_W(e+2) -> h(e+1) ->
s(e+1) -> scores(e+1) -> attn+epilogue(e), so the PE streams matmuls for
expert e while ACT/DVE cook the score tensor for e+1 one iteration ahead.
All big matmuls are bf16 (1 cycle/row); s/scores go through an fp32
c-pair and fp16 score storage, keeping |score| error ~4e-3.
"""

import sys

import numpy as np

for _p in ("/opt/trn_rl_repo",):
    if _p not in sys.path:
        sys.path.append(_p)

B, N, D, E = 8, 1024, 512, 8
P = 128
NB = N // P  # 8 node blocks
DB = D // P  # 4 hidden blocks
SLOPE = 0.2
DVE_SCORE_JB = (1, 4, 7)  # score blocks whose leaky-relu runs on DVE, not ACT

_CACHE = {}


def _build():
    from contextlib import ExitStack

    import concourse.bass as bass
    import concourse.tile as tile
    from concourse import bacc, mybir
    from concourse.masks import make_identity

    f32 = mybir.dt.float32
    f16 = mybir.dt.float16
    bf16 = mybir.dt.bfloat16
    i32 = mybir.dt.int32
    AF = mybir.ActivationFunctionType
    OP = mybir.AluOpType
    ts = bass.ts

    nc = bacc.Bacc("TRN2", target_bir_lowering=False, debug=False, num_swdge_queues=4)

    x_d = nc.dram_tensor("x", [N, D], f32, kind="ExternalInput")
    adj_d = nc.dram_tensor("adj", [N, N], i32, kind="ExternalInput")
    gw_d = nc.dram_tensor("gate_W", [D, E], f32, kind="ExternalInput")
    gb_d = nc.dram_tensor("gate_b", [E], f32, kind="ExternalInput")
    W_d = nc.dram_tensor("W", [E, D, D], f32, kind="ExternalInput")
    asrc_d = nc.dram_tensor("a_src", [E, D], f32, kind="ExternalInput")
    adst_d = nc.dram_tensor("a_dst", [E, D], f32, kind="ExternalInput")
    out_d = nc.dram_tensor("out", [N, D], f32, kind="ExternalOutput")

    with tile.TileContext(nc) as tc, ExitStack() as ctx:
        const = ctx.enter_context(tc.tile_pool(name="const", bufs=1))
        xT_p = ctx.enter_context(tc.tile_pool(name="xT", bufs=1))
        adjT_p = ctx.enter_context(tc.tile_pool(name="adjT", bufs=1))
        gate_p = ctx.enter_context(tc.tile_pool(name="gate", bufs=1))
        acc_p = ctx.enter_context(tc.tile_pool(name="acc", bufs=1))
        # PSUM: "ps" ring 4 banks; "rows" 2 banks; "shared" 2 banks = 8 total
        ps_big = ctx.enter_context(tc.tile_pool(name="ps_big", bufs=4, space="PSUM"))
        ps_row = ctx.enter_context(tc.tile_pool(name="ps_row", bufs=1, space="PSUM"))
        ps_sh = ctx.enter_context(tc.tile_pool(name="ps_sh", bufs=1, space="PSUM"))

        ident = const.tile([P, P], f32)
        make_identity(nc, ident)
        ident_b = const.tile([P, P], bf16)
        nc.vector.tensor_copy(out=ident_b, in_=ident)
        ones_col_b = const.tile([P, 1], bf16)
        nc.vector.memset(ones_col_b, 1.0)
        ones_q = const.tile([P, P], bf16)  # all-ones; quad-base rows/cols
        nc.vector.memset(ones_q, 1.0)

        gwsb = const.tile([P, DB, E], f32)
        nc.sync.dma_start(gwsb, gw_d.ap().rearrange("(db p) e -> p db e", p=P))
        gwsb_b = const.tile([P, DB, E], bf16)
        nc.vector.tensor_copy(out=gwsb_b, in_=gwsb)
        gbsb = const.tile([1, E], f32)
        nc.sync.dma_start(gbsb, gb_d.ap().rearrange("(o e) -> o e", o=1))
        gbsb_b = const.tile([1, E], bf16)
        nc.vector.tensor_copy(out=gbsb_b, in_=gbsb)

        xT_b = xT_p.tile([P, DB, N], bf16)  # xT_b[p, db, n] = x[n, db*128+p]
        adjT = adjT_p.tile([P, NB, N], bf16)  # adjT[p, jb, i] = adj[i, jb*128+p]
        gate = gate_p.tile([P, NB, E], f32)  # gate[p, nb, e]
        acc_t = acc_p.tile([P, NB, D], f32)  # MoE-combine accumulator

        # ---- stage 0: x transpose, gate softmax, adj transpose -------------
        with (
            tc.tile_pool(name="xin", bufs=3) as xin,
            tc.tile_pool(name="adjin", bufs=2) as adjin,
            tc.tile_pool(name="adjf", bufs=2) as adjf_p,
        ):
            for nb in range(NB):
                xt = xin.tile([P, D], f32, tag="xin")
                nc.sync.dma_start(xt, x_d[ts(nb, P), :])
                xt_b = xin.tile([P, D], bf16, tag="xin_b")
                if nb % 2 == 0:
                    nc.vector.tensor_copy(out=xt_b, in_=xt)
                else:
                    nc.scalar.copy(out=xt_b, in_=xt)
                pst = ps_big.tile([P, 512], bf16, tag="ps")
                for db in range(DB):
                    nc.tensor.transpose(pst[:, ts(db, P)], xt_b[:, ts(db, P)], ident_b)
                src = pst[:, 0:D].rearrange("p (db q) -> p db q", q=P)
                nc.vector.tensor_copy(out=xT_b[:, :, ts(nb, P)], in_=src)

            for nb in range(NB):
                psg = ps_big.tile([P, 512], f32, tag="ps")
                for db in range(DB):
                    nc.tensor.matmul(
                        psg[:, 0:E],
                        xT_b[:, db, ts(nb, P)],
                        gwsb_b[:, db, :],
                        start=(db == 0),
                        stop=False,
                    )
                nc.tensor.matmul(
                    psg[:, 0:E], ones_q[0:1, :], gbsb_b, start=False, stop=True
                )
                eg = const.tile([P, E], f32, tag=f"eg{nb % 2}")
                sg = const.tile([P, 1], f32, tag=f"sg{nb % 2}")
                nc.scalar.activation(eg, psg[:, 0:E], AF.Exp, accum_out=sg)
                rg0 = const.tile([P, 1], f32, tag=f"rg0{nb % 2}")
                nc.vector.reciprocal(rg0, sg)
                nc.vector.tensor_scalar(gate[:, nb, :], eg, rg0, None, OP.mult)

            for ib in range(NB):
                at = adjin.tile([P, N], i32, tag="adjin")
                nc.sync.dma_start(at, adj_d[ts(ib, P), :])
                af = adjf_p.tile([P, N], bf16, tag="adjf")
                if ib % 2 == 0:
                    nc.vector.tensor_copy(out=af, in_=at)
                else:
                    nc.scalar.copy(out=af, in_=at)
                for half in range(2):
                    pst = ps_big.tile([P, 512], bf16, tag="ps")
                    for k in range(4):
                        jb = half * 4 + k
                        nc.tensor.transpose(pst[:, ts(k, P)], af[:, ts(jb, P)], ident_b)
                    src = pst.rearrange("p (jb q) -> p jb q", q=P)
                    dst = adjT[:, half * 4 : (half + 1) * 4, ts(ib, P)]
                    nc.vector.tensor_copy(out=dst, in_=src)

        # ---- expert pipeline ------------------------------------------------
        W_p = ctx.enter_context(tc.tile_pool(name="W", bufs=3))
        apair_p = ctx.enter_context(tc.tile_pool(name="apair", bufs=3))
        h_p = ctx.enter_context(tc.tile_pool(name="h", bufs=2))
        pmT_p = ctx.enter_context(tc.tile_pool(name="pmT", bufs=16))
        t_p = ctx.enter_context(tc.tile_pool(name="tsc", bufs=2))
        p_p = ctx.enter_context(tc.tile_pool(name="psc", bufs=3))
        bcs_p = ctx.enter_context(tc.tile_pool(name="bcs", bufs=2))
        rows_p = ctx.enter_context(tc.tile_pool(name="rows_sb", bufs=2))
        scol_p = ctx.enter_context(tc.tile_pool(name="scol", bufs=2))
        elu_p = ctx.enter_context(tc.tile_pool(name="elu", bufs=2))
        tiny_p = ctx.enter_context(tc.tile_pool(name="tiny", bufs=4))
        outw_p = ctx.enter_context(tc.tile_pool(name="outw", bufs=2))

        state = {}  # per-expert live tiles

        def prep_w(e):
            """Load W/a for expert e; c_pair[d, v] = sum_z W[d,z] a_v[z]."""
            st = state.setdefault(e, {})
            Wt = W_p.tile([P, DB, D], f32, tag="W")
            nc.sync.dma_start(Wt, W_d[e].rearrange("(db p) z -> p db z", p=P))
            Wt_b = W_p.tile([P, DB, D], bf16, tag="Wb")
            for db in range(DB):
                if db % 2 == 0:
                    nc.vector.tensor_copy(out=Wt_b[:, db, :], in_=Wt[:, db, :])
                else:
                    nc.scalar.copy(out=Wt_b[:, db, :], in_=Wt[:, db, :])
            arow = apair_p.tile([1, 2 * D], f32, tag="arow")
            nc.sync.dma_start(arow[:, 0:D], asrc_d[e].rearrange("(o z) -> o z", o=1))
            nc.sync.dma_start(
                arow[:, D : 2 * D], adst_d[e].rearrange("(o z) -> o z", o=1)
            )
            absf = apair_p.tile([P, 2 * D], f32, tag="absf")
            nc.gpsimd.partition_broadcast(absf, arow)
            ap_t = apair_p.tile([P, DB, 2], f32, tag="cpair")
            for db in range(DB):
                for v in range(2):
                    scr = apair_p.tile([P, D], f32, tag="cw_scratch")
                    nc.gpsimd.tensor_mul(
                        scr, Wt[:, db, :], absf[:, v * D : (v + 1) * D]
                    )
                    nc.vector.reduce_sum(
                        ap_t[:, db, v : v + 1], scr, axis=mybir.AxisListType.X
                    )
            ap_t_b = apair_p.tile([P, DB, 2], bf16, tag="cpair_b")
            nc.vector.tensor_copy(out=ap_t_b, in_=ap_t)
            st["Wt_b"] = Wt_b
            st["ap_t_b"] = ap_t_b

        def stage_h(e):
            st = state[e]
            h_t = h_p.tile([P, NB, D], bf16, tag="h")
            for nb in range(NB):
                ph = ps_big.tile([P, 512], f32, tag="ps")
                for db in range(DB):
                    nc.tensor.matmul(
                        ph,
                        xT_b[:, db, ts(nb, P)],
                        st["Wt_b"][:, db, :],
                        start=(db == 0),
                        stop=(db == DB - 1),
                    )
                if nb % 2 == 0:
                    nc.vector.tensor_copy(out=h_t[:, nb, :], in_=ph)
                else:
                    nc.scalar.copy(out=h_t[:, nb, :], in_=ph)
            st["h_t"] = h_t

        def stage_s(e):
            """s rows (s_src at partition 0, s_dst at partition 32 of one
            PSUM bank pair), then sdcol column form + s_src partition
            broadcast for the score pass."""
            st = state[e]
            rows = ps_row.tile([P, N], f32, tag="rows")
            for v, prow in ((0, rows[0:1, :]), (1, rows[32:33, :])):
                for half in range(2):
                    for db in range(DB):
                        nc.tensor.matmul(
                            prow[:, half * 512 : (half + 1) * 512],
                            st["ap_t_b"][:, db, v : v + 1],
                            xT_b[:, db, half * 512 : (half + 1) * 512],
                            start=(db == 0),
                            stop=(db == DB - 1),
                        )
            rows_sb = rows_p.tile([P, N], bf16, tag="rows_sb")
            nc.vector.tensor_copy(out=rows_sb[0:1, :], in_=rows[0:1, :])
            nc.scalar.copy(out=rows_sb[32:33, :], in_=rows[32:33, :])
            # s_dst -> per-partition column form [128, NB] via tiny matmuls
            psd = ps_big.tile([P, 512], f32, tag="ps")
            for nb in range(NB):
                nc.tensor.matmul(
                    psd[:, nb : nb + 1],
                    rows_sb[32:33, ts(nb, P)],
                    ones_q[32:33, 0:1],
                    start=True,
                    stop=True,
                )
            sdcol = scol_p.tile([P, NB], f32, tag="sdcol")
            nc.vector.tensor_copy(out=sdcol, in_=psd[:, 0:NB])
            # broadcast s_src across partitions, then stash as fp16 in SBUF
            bc = ps_sh.tile([P, N], f32, tag="sh")
            nc.tensor.matmul(
                bc[:, 0:512], ones_q[0:1, :], rows_sb[0:1, 0:512], start=True, stop=True
            )
            nc.tensor.matmul(
                bc[:, 512:1024],
                ones_q[0:1, :],
                rows_sb[0:1, 512:1024],
                start=True,
                stop=True,
            )
            bcs = bcs_p.tile([P, N], f16, tag="bcs")
            nc.vector.tensor_copy(out=bcs[:, 0:512], in_=bc[:, 0:512])
            nc.scalar.copy(out=bcs[:, 512:1024], in_=bc[:, 512:1024])
            st["sdcol"] = sdcol
            st["bcs"] = bcs

        def stage_scores(e):
            st = state[e]
            sdcol, bcs = st["sdcol"], st["bcs"]
            pm_tiles = []
            for jb in range(NB):
                if jb in DVE_SCORE_JB:
                    t1 = t_p.tile([P, N], f16, tag="t1")
                    nc.vector.tensor_scalar(
                        t1, bcs, sdcol[:, jb : jb + 1], None, OP.add
                    )
                    sc = t_p.tile([P, N], f16, tag="sc")
                    nc.vector.scalar_tensor_tensor(
                        out=sc, in0=t1, scalar=SLOPE, in1=t1, op0=OP.mult, op1=OP.max
                    )
                else:
                    sc = t_p.tile([P, N], f16, tag="sc")
                    nc.scalar.activation(
                        sc, bcs, AF.Prelu, bias=sdcol[:, jb : jb + 1], alpha=SLOPE
                    )
                psc = p_p.tile([P, N], bf16, tag="p")
                nc.scalar.activation(psc, sc, AF.Exp)
                pm = pmT_p.tile([P, N], bf16, tag="pmT")
                nc.vector.tensor_mul(pm, psc, adjT[:, jb, :])
                pm_tiles.append(pm)
            st["pm"] = pm_tiles

        def stage_S(e):
            """Masked-softmax denominators for expert e from its pm tiles."""
            st = state[e]
            pm_tiles = st["pm"]
            psS = ps_sh.tile([P, N], f32, tag="sh")
            for half in range(2):
                for jb in range(NB):
                    nc.tensor.matmul(
                        psS[0:1, half * 512 : (half + 1) * 512],
                        ones_col_b,
                        pm_tiles[jb][:, half * 512 : (half + 1) * 512],
                        start=(jb == 0),
                        stop=(jb == NB - 1),
                    )
            srow_sb = rows_p.tile([1, N], bf16, tag="Srow")
            nc.vector.tensor_copy(out=srow_sb, in_=psS[0:1, :])
            psc2 = ps_big.tile([P, 512], f32, tag="ps")
            for nb in range(NB):
                nc.tensor.matmul(
                    psc2[:, nb : nb + 1],
                    srow_sb[0:1, ts(nb, P)],
                    ones_q[0:1, 0:1],
                    start=True,
                    stop=True,
                )
            rS8 = tiny_p.tile([P, NB], f32, tag="rS8")
            nc.vector.reciprocal(rS8, psc2[:, 0:NB])
            st["rS8"] = rS8

        def stage_attn(e):
            st = state[e]
            pm_tiles, h_t, rS8 = st["pm"], st["h_t"], st["rS8"]
            for ib in range(NB):
                po = ps_big.tile([P, 512], f32, tag="ps")
                for jb in range(NB):
                    nc.tensor.matmul(
                        po,
                        pm_tiles[jb][:, ts(ib, P)],
                        h_t[:, jb, :],
                        start=(jb == 0),
                        stop=(jb == NB - 1),
                    )
                rg = tiny_p.tile([P, 1], f32, tag="rg")
                nc.vector.tensor_scalar(
                    rg, rS8[:, ib : ib + 1], gate[:, ib, e : e + 1], None, OP.mult
                )
                r_sb = elu_p.tile([P, D], f32, tag="r")
                nc.scalar.activation(r_sb, po, AF.Relu, scale=rg)
                m_sb = elu_p.tile([P, D], f32, tag="n")
                nc.vector.tensor_scalar(
                    m_sb, po, rS8[:, ib : ib + 1], 0.0, OP.mult, OP.min
                )
                e_sb = elu_p.tile([P, D], f32, tag="e2")
                nc.scalar.activation(e_sb, m_sb, AF.Exp)

                if e == 0:
                    # acc = g*e2 + r   (r already carries g via the rg scale)
                    nc.vector.scalar_tensor_tensor(
                        out=acc_t[:, ib, :],
                        in0=e_sb,
                        scalar=gate[:, ib, e : e + 1],
                        in1=r_sb,
                        op0=OP.mult,
                        op1=OP.add,
                    )
                else:
                    nc.vector.scalar_tensor_tensor(
                        out=acc_t[:, ib, :],
                        in0=e_sb,
                        scalar=gate[:, ib, e : e + 1],
                        in1=acc_t[:, ib, :],
                        op0=OP.mult,
                        op1=OP.add,
                    )
                    nc.gpsimd.tensor_add(acc_t[:, ib, :], acc_t[:, ib, :], r_sb)
            del state[e]["pm"], state[e]["h_t"]

        # pipeline: scores run one expert ahead of the attention matmuls
        prep_w(0)
        prep_w(1)
        stage_h(0)
        stage_s(0)
        stage_scores(0)
        for e in range(E):
            stage_S(e)
            if e + 2 < E:
                prep_w(e + 2)
            if e + 1 < E:
                stage_h(e + 1)
                stage_s(e + 1)
                stage_scores(e + 1)
            stage_attn(e)

        # ---- writeback -----------------------------------------------------
        for ib in range(NB):
            ow = outw_p.tile([P, D], f32, tag="ow")
            nc.vector.tensor_scalar(ow, acc_t[:, ib, :], 1.0, None, OP.subtract)
            nc.sync.dma_start(out_d[ts(ib, P), :], ow)

    nc.compile()
    return nc


def _get_nc():
    if "nc" not in _CACHE:
        _CACHE["nc"] = _build()
    return _CACHE["nc"]


def _reset_device():
    # Defensive: clear any wedged accelerator state left by a prior process.
    try:
        import ctypes

        import jax

        jax.devices()
        lib = ctypes.CDLL("/opt/axon/libaxon_pjrt.so")
        lib.axon_reset.restype = ctypes.c_int64
        lib.axon_reset()
    except Exception:
        pass


def _run(inputs: dict, trace: bool = False):
    from concourse.bass_utils import run_bass_kernel_spmd

    _reset_device()
    nc = _get_nc()
    in_maps = []
    for c in range(8):
        in_maps.append(
            {
                "x": np.ascontiguousarray(inputs["x"][c], dtype=np.float32),
                "adj": np.ascontiguousarray(inputs["adj"][c], dtype=np.int32),
                "gate_W": np.ascontiguousarray(inputs["gate_W"], dtype=np.float32),
                "gate_b": np.ascontiguousarray(inputs["gate_b"], dtype=np.float32),
                "W": np.ascontiguousarray(inputs["W"], dtype=np.float32),
                "a_src": np.ascontiguousarray(inputs["a_src"], dtype=np.float32),
                "a_dst": np.ascontiguousarray(inputs["a_dst"], dtype=np.float32),
            }
        )
    res = run_bass_kernel_spmd(nc, in_maps, list(range(8)), trace=trace)
    out = np.stack([res.results[c]["out"] for c in range(8)], axis=0)
    return out.astype(np.float32), res


def kernel(**inputs) -> np.ndarray:
    out, _ = _run(inputs, trace=False)
    return out


def kernel_traced(**inputs):
    out, res = _run(inputs, trace=True)
    try:
        if res.instructions_and_trace:
            print("trace path:", res.instructions_and_trace[1])
        if res.profile_json:
            print("profile json:", res.profile_json)
    except Exception:
        pass
    return out, res.exec_time_ns


# revision 7
# speedup vs baseline: 1.3150x; 1.2056x over previous
"""MoE-GAT kernel for Trainium2 (Bass/Tile), SPMD over 8 NeuronCores.

Sharding: data-parallel over batch (B=8 -> 1 batch element per core).
Each core receives its x[b] / adj[b] slice plus the full shared weights
and computes out[b] = MoEGAT(x[b], adj[b]) independently (no collectives).

Math per core (N=1024 nodes, D=512 hidden, E=8 experts):
  gate = softmax(x @ gate_W + gate_b)                  [N, E]
  h_e  = x @ W[e]                                      [N, D]
  s_src = h_e @ a_src[e] ; s_dst = h_e @ a_dst[e]      [N]
  scoresT[j, i] = leaky_relu(s_src[i] + s_dst[j], .2)  (transposed layout)
  pmT[j, i] = exp(scoresT) * adj[i, j]
  out_e[i, :] = (pmT[:, i] . h_e) / S_i ; S_i = sum_j pmT[j, i]
  out[i] = sum_e gate[i, e] * elu(out_e[i]) = acc - 1, using sum_e gate = 1
  elu(v) = relu(v) + exp(min(v, 0)) - 1

Software pipeline per iteration e:
  S(e) -> load W(e+2) -> c_pair(e+1) -> h(e+1) -> s(e+1) -> scores(e+1)
  -> attn+elu+combine(e)
so the PE streams matmuls for expert e while ACT cooks the score tensor
for e+1. c_pair = W @ a runs on the PE against an XBAR-DMA-transposed
W (a carried as a bf16 hi+lo pair so only W's bf16 rounding remains),
and the ELU combine accumulates on the (otherwise idle) Pool engine.
"""

import sys

import numpy as np

for _p in ("/opt/trn_rl_repo",):
    if _p not in sys.path:
        sys.path.append(_p)

B, N, D, E = 8, 1024, 512, 8
P = 128
NB = N // P  # 8 node blocks
DB = D // P  # 4 hidden blocks
SLOPE = 0.2

_CACHE = {}


def _build():
    from contextlib import ExitStack

    import concourse.bass as bass
    import concourse.tile as tile
    from concourse import bacc, mybir
    from concourse.masks import make_identity

    f32 = mybir.dt.float32
    f16 = mybir.dt.float16
    bf16 = mybir.dt.bfloat16
    i32 = mybir.dt.int32
    AF = mybir.ActivationFunctionType
    OP = mybir.AluOpType
    ts = bass.ts

    nc = bacc.Bacc("TRN2", target_bir_lowering=False, debug=False, num_swdge_queues=4)

    x_d = nc.dram_tensor("x", [N, D], f32, kind="ExternalInput")
    adj_d = nc.dram_tensor("adj", [N, N], i32, kind="ExternalInput")
    gw_d = nc.dram_tensor("gate_W", [D, E], f32, kind="ExternalInput")
    gb_d = nc.dram_tensor("gate_b", [E], f32, kind="ExternalInput")
    W_d = nc.dram_tensor("W", [E, D, D], f32, kind="ExternalInput")
    asrc_d = nc.dram_tensor("a_src", [E, D], f32, kind="ExternalInput")
    adst_d = nc.dram_tensor("a_dst", [E, D], f32, kind="ExternalInput")
    out_d = nc.dram_tensor("out", [N, D], f32, kind="ExternalOutput")

    with tile.TileContext(nc) as tc, ExitStack() as ctx:
        const = ctx.enter_context(tc.tile_pool(name="const", bufs=1))
        xT_p = ctx.enter_context(tc.tile_pool(name="xT", bufs=1))
        adjT_p = ctx.enter_context(tc.tile_pool(name="adjT", bufs=1))
        gate_p = ctx.enter_context(tc.tile_pool(name="gate", bufs=1))
        acc_p = ctx.enter_context(tc.tile_pool(name="acc", bufs=1))
        # PSUM: "ps2" ring 3x[P,1024] = 6 banks; "sh" 1x[P,1024] = 2 banks
        ps_big = ctx.enter_context(tc.tile_pool(name="ps_big", bufs=3, space="PSUM"))
        ps_sh = ctx.enter_context(tc.tile_pool(name="ps_sh", bufs=1, space="PSUM"))

        ident = const.tile([P, P], f32)
        make_identity(nc, ident)
        ident_b = const.tile([P, P], bf16)
        nc.vector.tensor_copy(out=ident_b, in_=ident)
        ones_col_b = const.tile([P, 1], bf16)
        nc.vector.memset(ones_col_b, 1.0)
        ones_q = const.tile([P, P], bf16)  # all-ones; quad-base rows/cols
        nc.vector.memset(ones_q, 1.0)

        gwsb = const.tile([P, DB, E], f32)
        nc.sync.dma_start(gwsb, gw_d.ap().rearrange("(db p) e -> p db e", p=P))
        gwsb_b = const.tile([P, DB, E], bf16)
        nc.vector.tensor_copy(out=gwsb_b, in_=gwsb)
        gbsb = const.tile([1, E], f32)
        nc.sync.dma_start(gbsb, gb_d.ap().rearrange("(o e) -> o e", o=1))
        gbsb_b = const.tile([1, E], bf16)
        nc.vector.tensor_copy(out=gbsb_b, in_=gbsb)

        xT_b = xT_p.tile([P, DB, N], bf16)  # xT_b[p, db, n] = x[n, db*128+p]
        adjT = adjT_p.tile([P, NB, N], bf16)  # adjT[p, jb, i] = adj[i, jb*128+p]
        gate = gate_p.tile([P, NB, E], f32)  # gate[p, nb, e]
        acc_t = acc_p.tile([P, NB, D], f32)  # MoE-combine accumulator

        # ---- stage 0: x transpose, gate softmax, adj transpose -------------
        with (
            tc.tile_pool(name="xin", bufs=3) as xin,
            tc.tile_pool(name="adjin", bufs=2) as adjin,
            tc.tile_pool(name="adjf", bufs=2) as adjf_p,
        ):
            for nb in range(NB):
                xt = xin.tile([P, D], f32, tag="xin")
                nc.sync.dma_start(xt, x_d[ts(nb, P), :])
                xt_b = xin.tile([P, D], bf16, tag="xin_b")
                if nb % 2 == 0:
                    nc.vector.tensor_copy(out=xt_b, in_=xt)
                else:
                    nc.scalar.copy(out=xt_b, in_=xt)
                pst = ps_big.tile([P, 512], bf16, tag="ps")
                for db in range(DB):
                    nc.tensor.transpose(pst[:, ts(db, P)], xt_b[:, ts(db, P)], ident_b)
                src = pst[:, 0:D].rearrange("p (db q) -> p db q", q=P)
                nc.vector.tensor_copy(out=xT_b[:, :, ts(nb, P)], in_=src)

            for nb in range(NB):
                psg = ps_big.tile([P, 512], f32, tag="ps")
                for db in range(DB):
                    nc.tensor.matmul(
                        psg[:, 0:E],
                        xT_b[:, db, ts(nb, P)],
                        gwsb_b[:, db, :],
                        start=(db == 0),
                        stop=False,
                    )
                nc.tensor.matmul(
                    psg[:, 0:E], ones_q[0:1, :], gbsb_b, start=False, stop=True
                )
                eg = const.tile([P, E], f32, tag=f"eg{nb % 2}")
                sg = const.tile([P, 1], f32, tag=f"sg{nb % 2}")
                nc.scalar.activation(eg, psg[:, 0:E], AF.Exp, accum_out=sg)
                rg0 = const.tile([P, 1], f32, tag=f"rg0{nb % 2}")
                nc.vector.reciprocal(rg0, sg)
                nc.vector.tensor_scalar(gate[:, nb, :], eg, rg0, None, OP.mult)

            for ib in range(NB):
                at = adjin.tile([P, N], i32, tag="adjin")
                nc.sync.dma_start(at, adj_d[ts(ib, P), :])
                af = adjf_p.tile([P, N], bf16, tag="adjf")
                if ib % 2 == 0:
                    nc.vector.tensor_copy(out=af, in_=at)
                else:
                    nc.scalar.copy(out=af, in_=at)
                for half in range(2):
                    pst = ps_big.tile([P, 512], bf16, tag="ps")
                    for k in range(4):
                        jb = half * 4 + k
                        nc.tensor.transpose(pst[:, ts(k, P)], af[:, ts(jb, P)], ident_b)
                    src = pst.rearrange("p (jb q) -> p jb q", q=P)
                    dst = adjT[:, half * 4 : (half + 1) * 4, ts(ib, P)]
                    nc.vector.tensor_copy(out=dst, in_=src)

        # ---- expert pipeline ------------------------------------------------
        W_p = ctx.enter_context(tc.tile_pool(name="W", bufs=2))
        apair_p = ctx.enter_context(tc.tile_pool(name="apair", bufs=2))
        h_p = ctx.enter_context(tc.tile_pool(name="h", bufs=2))
        pmT_p = ctx.enter_context(tc.tile_pool(name="pmT", bufs=16))
        t_p = ctx.enter_context(tc.tile_pool(name="tsc", bufs=2))
        p_p = ctx.enter_context(tc.tile_pool(name="psc", bufs=3))
        rows_p = ctx.enter_context(tc.tile_pool(name="rows_sb", bufs=2))
        scol_p = ctx.enter_context(tc.tile_pool(name="scol", bufs=2))
        elu_p = ctx.enter_context(tc.tile_pool(name="elu", bufs=2))
        tiny_p = ctx.enter_context(tc.tile_pool(name="tiny", bufs=4))
        outw_p = ctx.enter_context(tc.tile_pool(name="outw", bufs=2))

        state = {}  # per-expert live tiles

        def prep_w(e):
            """Load W/a for expert e; bf16-cast W; XBAR-transpose W;
            land a_src/a_dst as bf16 hi+lo column pairs."""
            st = state.setdefault(e, {})
            Wt = W_p.tile([P, DB, D], f32, tag="W")
            nc.sync.dma_start(Wt, W_d[e].rearrange("(db p) z -> p db z", p=P))
            Wt_b = W_p.tile([P, DB, D], bf16, tag="Wb")
            for db in range(DB):
                nc.vector.tensor_copy(out=Wt_b[:, db, :], in_=Wt[:, db, :])
            WT_b = W_p.tile([P, DB, D], bf16, tag="WTb")  # [z-part, zb, d]
            for db in range(DB):
                for zb in range(DB):
                    nc.sync.dma_start_transpose(
                        WT_b[:, zb, ts(db, P)], Wt_b[:, db, ts(zb, P)]
                    )
            a_colf = apair_p.tile([P, 2, DB], f32, tag="acolf")
            nc.sync.dma_start(
                a_colf[:, 0, :], asrc_d[e].rearrange("(zb p) -> p zb", p=P)
            )
            nc.sync.dma_start(
                a_colf[:, 1, :], adst_d[e].rearrange("(zb p) -> p zb", p=P)
            )
            a_hi = apair_p.tile([P, 2, DB], bf16, tag="ahi")
            nc.vector.tensor_copy(out=a_hi, in_=a_colf)
            a_lo = apair_p.tile([P, 2, DB], bf16, tag="alo")
            nc.vector.scalar_tensor_tensor(
                out=a_lo, in0=a_colf, scalar=1.0, in1=a_hi, op0=OP.mult, op1=OP.subtract
            )
            st["Wt_b"] = Wt_b
            st["WT_b"] = WT_b
            st["a_hi"] = a_hi
            st["a_lo"] = a_lo

        def prep_c(e):
            """c_pair[d, v] = sum_z W[d, z] a_v[z] on the PE (W transposed)."""
            st = state[e]
            WT_b = st["WT_b"]
            psc_pair = ps_sh.tile([P, N], f32, tag="sh")
            for db in range(DB):
                for pi, at in enumerate((st["a_hi"], st["a_lo"])):
                    for zb in range(DB):
                        nc.tensor.matmul(
                            psc_pair[:, 2 * db : 2 * db + 2],
                            WT_b[:, zb, ts(db, P)],
                            at[:, :, zb],
                            start=(pi == 0 and zb == 0),
                            stop=(pi == 1 and zb == DB - 1),
                        )
            ap_t_b = apair_p.tile([P, DB, 2], bf16, tag="cpair_b")
            nc.vector.tensor_copy(
                out=ap_t_b,
                in_=psc_pair[:, 0 : 2 * DB].rearrange("p (db v) -> p db v", v=2),
            )
            st["ap_t_b"] = ap_t_b

        def stage_h(e):
            st = state[e]
            h_t = h_p.tile([P, NB, D], bf16, tag="h")
            for pair in range(NB // 2):
                ph = ps_big.tile([P, 1024], f32, tag="ps")
                for half in range(2):
                    nb = 2 * pair + half
                    for db in range(DB):
                        nc.tensor.matmul(
                            ph[:, half * 512 : (half + 1) * 512],
                            xT_b[:, db, ts(nb, P)],
                            st["Wt_b"][:, db, :],
                            start=(db == 0),
                            stop=(db == DB - 1),
                        )
                nc.vector.tensor_copy(
                    out=h_t[:, 2 * pair : 2 * pair + 2, :],
                    in_=ph.rearrange("p (nb z) -> p nb z", z=D),
                )
            st["h_t"] = h_t

        def stage_s(e):
            """s rows (s_src at partition 0, s_dst at partition 32 of one
            PSUM bank pair), then sdcol column form + s_src partition
            broadcast; the score Prelu reads the broadcast from PSUM."""
            st = state[e]
            rows = ps_sh.tile([P, N], f32, tag="sh")
            for v, prow in ((0, rows[0:1, :]), (1, rows[32:33, :])):
                for half in range(2):
                    for db in range(DB):
                        nc.tensor.matmul(
                            prow[:, half * 512 : (half + 1) * 512],
                            st["ap_t_b"][:, db, v : v + 1],
                            xT_b[:, db, half * 512 : (half + 1) * 512],
                            start=(db == 0),
                            stop=(db == DB - 1),
                        )
            rows_sb = rows_p.tile([P, N], bf16, tag="rows_sb")
            nc.vector.tensor_copy(out=rows_sb[0:1, :], in_=rows[0:1, :])
            nc.scalar.copy(out=rows_sb[32:33, :], in_=rows[32:33, :])
            # s_dst -> per-partition column form [128, NB] via tiny matmuls
            psd = ps_sh.tile([P, 512], f32, tag="sh")
            for nb in range(NB):
                nc.tensor.matmul(
                    psd[:, nb : nb + 1],
                    rows_sb[32:33, ts(nb, P)],
                    ones_q[32:33, 0:1],
                    start=True,
                    stop=True,
                )
            sdcol = scol_p.tile([P, NB], f32, tag="sdcol")
            nc.vector.tensor_copy(out=sdcol, in_=psd[:, 0:NB])
            # broadcast s_src across partitions; scores read it from PSUM
            bc = ps_sh.tile([P, N], f32, tag="sh")
            nc.tensor.matmul(
                bc[:, 0:512], ones_q[0:1, :], rows_sb[0:1, 0:512], start=True, stop=True
            )
            nc.tensor.matmul(
                bc[:, 512:1024],
                ones_q[0:1, :],
                rows_sb[0:1, 512:1024],
                start=True,
                stop=True,
            )
            st["sdcol"] = sdcol
            st["bc"] = bc

        def stage_scores(e):
            st = state[e]
            sdcol, bc = st["sdcol"], st["bc"]
            pm_tiles = []
            for jb in range(NB):
                sc = t_p.tile([P, N], f16, tag="sc")
                nc.scalar.activation(
                    sc, bc, AF.Prelu, bias=sdcol[:, jb : jb + 1], alpha=SLOPE
                )
                psc = p_p.tile([P, N], bf16, tag="p")
                nc.scalar.activation(psc, sc, AF.Exp)
                pm = pmT_p.tile([P, N], bf16, tag="pmT")
                nc.vector.tensor_mul(pm, psc, adjT[:, jb, :])
                pm_tiles.append(pm)
            st["pm"] = pm_tiles
            del st["bc"]

        def stage_S(e):
            """Masked-softmax denominators for expert e from its pm tiles."""
            st = state[e]
            pm_tiles = st["pm"]
            psS = ps_sh.tile([P, N], f32, tag="sh")
            for half in range(2):
                for jb in range(NB):
                    nc.tensor.matmul(
                        psS[0:1, half * 512 : (half + 1) * 512],
                        ones_col_b,
                        pm_tiles[jb][:, half * 512 : (half + 1) * 512],
                        start=(jb == 0),
                        stop=(jb == NB - 1),
                    )
            srow_sb = rows_p.tile([1, N], bf16, tag="Srow")
            nc.vector.tensor_copy(out=srow_sb, in_=psS[0:1, :])
            psc2 = ps_sh.tile([P, 512], f32, tag="sh")
            for nb in range(NB):
                nc.tensor.matmul(
                    psc2[:, nb : nb + 1],
                    srow_sb[0:1, ts(nb, P)],
                    ones_q[0:1, 0:1],
                    start=True,
                    stop=True,
                )
            rS8 = tiny_p.tile([P, NB], f32, tag="rS8")
            nc.vector.reciprocal(rS8, psc2[:, 0:NB])
            st["rS8"] = rS8

        def stage_attn(e):
            st = state[e]
            pm_tiles, h_t, rS8 = st["pm"], st["h_t"], st["rS8"]
            for pair in range(NB // 2):
                po = ps_big.tile([P, 1024], f32, tag="ps")
                for half in range(2):
                    ib = 2 * pair + half
                    pos = po[:, half * 512 : (half + 1) * 512]
                    for jb in range(NB):
                        nc.tensor.matmul(
                            pos,
                            pm_tiles[jb][:, ts(ib, P)],
                            h_t[:, jb, :],
                            start=(jb == 0),
                            stop=(jb == NB - 1),
                        )
                for half in range(2):
                    ib = 2 * pair + half
                    pos = po[:, half * 512 : (half + 1) * 512]
                    rg = tiny_p.tile([P, 1], f32, tag="rg")
                    nc.vector.tensor_scalar(
                        rg, rS8[:, ib : ib + 1], gate[:, ib, e : e + 1], None, OP.mult
                    )
                    r_sb = elu_p.tile([P, D], f32, tag="r")
                    if ib % 2 == 0:
                        nc.scalar.activation(r_sb, pos, AF.Relu, scale=rg)
                    else:
                        nc.vector.tensor_scalar(r_sb, pos, rg, 0.0, OP.mult, OP.max)
                    m_sb = elu_p.tile([P, D], f32, tag="n")
                    nc.vector.tensor_scalar(
                        m_sb, pos, rS8[:, ib : ib + 1], 0.0, OP.mult, OP.min
                    )
                    e_sb = elu_p.tile([P, D], f32, tag="e2")
                    nc.scalar.activation(e_sb, m_sb, AF.Exp)

                    if e == 0:
                        # acc = g*e2 + r  (r already carries g via rg)
                        nc.vector.scalar_tensor_tensor(
                            out=acc_t[:, ib, :],
                            in0=e_sb,
                            scalar=gate[:, ib, e : e + 1],
                            in1=r_sb,
                            op0=OP.mult,
                            op1=OP.add,
                        )
                    else:
                        nc.vector.scalar_tensor_tensor(
                            out=acc_t[:, ib, :],
                            in0=e_sb,
                            scalar=gate[:, ib, e : e + 1],
                            in1=acc_t[:, ib, :],
                            op0=OP.mult,
                            op1=OP.add,
                        )
                        nc.gpsimd.tensor_add(acc_t[:, ib, :], acc_t[:, ib, :], r_sb)
            del state[e]["pm"], state[e]["h_t"]

        # pipeline: scores run one expert ahead of the attention matmuls
        prep_w(0)
        prep_w(1)
        prep_c(0)
        stage_h(0)
        stage_s(0)
        stage_scores(0)
        for e in range(E):
            stage_S(e)
            if e + 2 < E:
                prep_w(e + 2)
            if e + 1 < E:
                prep_c(e + 1)
                stage_h(e + 1)
                stage_s(e + 1)
                stage_scores(e + 1)
            stage_attn(e)

        # ---- writeback -----------------------------------------------------
        for ib in range(NB):
            ow = outw_p.tile([P, D], f32, tag="ow")
            nc.vector.tensor_scalar(ow, acc_t[:, ib, :], 1.0, None, OP.subtract)
            nc.sync.dma_start(out_d[ts(ib, P), :], ow)

    nc.compile()
    return nc


def _get_nc():
    if "nc" not in _CACHE:
        _CACHE["nc"] = _build()
    return _CACHE["nc"]


def _reset_device():
    # Defensive: clear any wedged accelerator state left by a prior process.
    try:
        import ctypes

        import jax

        jax.devices()
        lib = ctypes.CDLL("/opt/axon/libaxon_pjrt.so")
        lib.axon_reset.restype = ctypes.c_int64
        lib.axon_reset()
    except Exception:
        pass


def _run(inputs: dict, trace: bool = False):
    from concourse.bass_utils import run_bass_kernel_spmd

    _reset_device()
    nc = _get_nc()
    in_maps = []
    for c in range(8):
        in_maps.append(
            {
                "x": np.ascontiguousarray(inputs["x"][c], dtype=np.float32),
                "adj": np.ascontiguousarray(inputs["adj"][c], dtype=np.int32),
                "gate_W": np.ascontiguousarray(inputs["gate_W"], dtype=np.float32),
                "gate_b": np.ascontiguousarray(inputs["gate_b"], dtype=np.float32),
                "W": np.ascontiguousarray(inputs["W"], dtype=np.float32),
                "a_src": np.ascontiguousarray(inputs["a_src"], dtype=np.float32),
                "a_dst": np.ascontiguousarray(inputs["a_dst"], dtype=np.float32),
            }
        )
    res = run_bass_kernel_spmd(nc, in_maps, list(range(8)), trace=trace)
    out = np.stack([res.results[c]["out"] for c in range(8)], axis=0)
    return out.astype(np.float32), res


def kernel(**inputs) -> np.ndarray:
    out, _ = _run(inputs, trace=False)
    return out


def kernel_traced(**inputs):
    out, res = _run(inputs, trace=True)
    try:
        if res.instructions_and_trace:
            print("trace path:", res.instructions_and_trace[1])
        if res.profile_json:
            print("profile json:", res.profile_json)
    except Exception:
        pass
    return out, res.exec_time_ns
